# revision 24
# baseline (speedup 1.0000x reference)
"""Causal self-attention (B=4, S=4096, D=64, H=4) on 8 TRN2 NeuronCores.

Sharding: the 16 (batch, head) pairs are distributed 2-per-core
(core c -> batch c//2, heads (2*(c%2), 2*(c%2)+1)). Each core runs the
full fused attention for its 2 pairs; no cross-core communication.

Two tricks carry the kernel:

1. BILINEAR SCORES (K=64 keeps the PE's HAM clock gate warm). The
   TRN2 PE clock-gates to 1.2 GHz unless the activity monitor sees
   high-K matmuls (measured: K=16 4-way row-tiled matmuls never warm
   the clock, K>=64 does). Instead of Q@K^T with its K=16 (head dim)
   contraction, fold the projections into a host-precomputed bilinear
   form G_h = Wk_aug^T Wq_aug / sqrt(Dh) [65,65]:
       scores^T = x_aug G x_aug^T = x @ (G[0:64,:] @ x_aug^T)
   The per-query row G[64,:]@... cancels in softmax and is dropped.
   On device: Z_p = G64_p @ x_aug^T (K=65), then each score block is
   x^T_block @ Z_p with K=64, 2-way row-tiled (tile_position 0/64) --
   every matmul in the kernel now registers as HAM-busy, holding
   2.4 GHz, and the Q/K projection pass disappears entirely.

2. TWO-ENGINE SOFTMAX EXP (breaks the ACT throughput wall: 18.9M
   exps/core at 1 elem/cycle/lane is ~123us on ACT alone). Diagonal
   (causal-masked) key blocks take exact ACT exp; off-diagonal blocks
   are greedily balanced between ACT exact exp and a one-instruction
   DVE Schraudolph exp emitting bf16 BITS directly:
       bf16_bits(exp(s)) ~= int16(184.665*s + B2)
   (tensor_scalar mult+add, f32 PSUM in -> int16 SBUF out, bitcast
   back to bf16 for the P@V matmul). Keeping diagonal blocks exact
   confines the ~3% Schraudolph weight error to wide softmax sums
   where it averages out (measured rel err 0.011, tolerance 2e-2)
   and keeps -1e4 masked scores away from int16 saturation.

Scores are computed TRANSPOSED (key position on partitions) so P@V
needs no transpose; the softmax denominator comes free from a 17th
all-ones column in V; the final division happens on host. Exp skips
max-subtraction (scores are O(13), f32 exp cannot overflow).
"""

import numpy as np
import ml_dtypes

_B, _S, _D = 4, 4096, 64
_H, _Dh = 4, 16
_NC = 8
_SCALE = 1.0 / np.sqrt(_Dh)
_MASK_NEG = -10000.0
_NQB = _S // 512  # 8 query super-blocks of 512
_NKB = _S // 128  # 32 key blocks of 128
_CHUNK = 2  # k-blocks per exp chunk (2 PSUM banks, 3 bufs)

# Schraudolph exp -> bf16 bits: i16 = A2*s + B2, bitcast to bf16
_EXP_A2 = (2.0**23 / np.log(2.0)) / 65536.0
_EXP_B2 = (127.0 * 2.0**23 - 366393.0) / 65536.0

_cache = {}


def _build_nc():
    import concourse.tile as tile
    from concourse import bacc, mybir

    bf = mybir.dt.bfloat16
    i16 = mybir.dt.int16
    f32 = mybir.dt.float32
    Exp = mybir.ActivationFunctionType.Exp
    Mult = mybir.AluOpType.mult
    Add = mybir.AluOpType.add

    nc = bacc.Bacc("TRN2", target_bir_lowering=False, debug=False, num_devices=_NC)
    xT_d = nc.dram_tensor("xT", [_D + 1, _S], bf, kind="ExternalInput").ap()
    gt_d = nc.dram_tensor("gt", [128, 128], bf, kind="ExternalInput").ap()
    gb_d = nc.dram_tensor("gb", [128, 1], f32, kind="ExternalInput").ap()
    wv_d = nc.dram_tensor("wv", [_D + 1, 32], bf, kind="ExternalInput").ap()
    mask_d = nc.dram_tensor("mask", [128, 128], f32, kind="ExternalInput").ap()
    out_d = nc.dram_tensor("out", [2, 17, _S], f32, kind="ExternalOutput").ap()

    # greedy ACT/DVE balance: estimated busy ns accumulated per engine
    load = {"act": 0.0, "dve": 0.0, "n": 0}
    ACT_NS, DVE_NS = 1e9 / 1.2e9, 1e9 / 0.96e9  # per lane-element
    OVH = 350.0  # ACT per-instruction overhead estimate
    OVHD = 230.0  # DVE per-instruction overhead estimate

    def pick(act_cost, dve_cost):
        if load["act"] + act_cost <= load["dve"] + dve_cost:
            load["act"] += act_cost
            return "act"
        load["dve"] += dve_cost
        return "dve"

    with tile.TileContext(nc) as tc:
        with tc.tile_pool(name="singles", bufs=1) as singles:
            xT = singles.tile([_D + 1, _S], bf, tag="xT")
            xTc = singles.tile([128, _S], bf, tag="xTc")
            gt = singles.tile([128, 128], bf, tag="gt")
            gb = singles.tile([128, 1], f32, tag="gb")
            wv = singles.tile([_D + 1, 32], bf, tag="wv")
            maskt = singles.tile([128, 128], f32, tag="mask")
            # gt/wv first (tiny, unblock Z/V matmuls), xT split across the
            # sync and scalar HWDGE queues (each partition row is one DMA
            # descriptor, so a single transfer serializes ~65 of them);
            # odd-row score copy of x^T replicated SBUF->SBUF on the
            # gpsimd DGE as each chunk lands.
            nc.sync.dma_start(out=gt[:], in_=gt_d)
            nc.sync.dma_start(out=gb[:], in_=gb_d)
            nc.scalar.dma_start(out=wv[:], in_=wv_d)
            # high xT columns first: the first attention unit is qi=7, so
            # the projection consumes chunks in descending order
            for i, c in enumerate((3, 2, 1, 0)):
                eng = nc.sync if i % 2 == 0 else nc.scalar
                eng.dma_start(
                    out=xT[:, 1024 * c : 1024 * (c + 1)],
                    in_=xT_d[:, 1024 * c : 1024 * (c + 1)],
                )
                nc.gpsimd.dma_start(
                    out=xTc[64:128, 1024 * c : 1024 * (c + 1)],
                    in_=xT[0:64, 1024 * c : 1024 * (c + 1)],
                )
            nc.scalar.dma_start(out=maskt[:], in_=mask_d)

            # Z_p replicated at partition offsets 0/64 for 2-way row-tiled
            # score matmuls (xTc comes replicated from host).
            Zt = [singles.tile([128, _S], bf, tag=f"Zt{p}", name=f"Zt{p}") for p in range(2)]
            V = [singles.tile([128, 17 * _NKB], bf, tag=f"V{p}", name=f"V{p}") for p in range(2)]
            for p in range(2):
                nc.vector.memset(V[p][:], 1.0)

            # ---- attention pools (opened early: the Z/V projection
            # draws its PSUM tiles from the same pools, so there is no
            # pool-boundary barrier between projection and attention) ----
            with (
                tc.tile_pool(name="ps_sc", bufs=3, space="PSUM") as ps_sc,
                tc.tile_pool(name="ps_o", bufs=2, space="PSUM") as ps_o,
                tc.tile_pool(name="ptp", bufs=2) as ptp,
                tc.tile_pool(name="stg", bufs=3) as stg,
            ):
                # ---- HAM warm-up burst ----
                # ~4.5us of dense K=128 matmuls on the (tiny, first-to-
                # arrive) gt tile while the PE would otherwise idle waiting
                # for the xT DMA: flips the clock gate to 2.4 GHz before
                # the real work starts. Output is never read.
                warm = ps_o.tile([128, 128], f32, tag="po", name="po")
                for _ in range(44):
                    nc.tensor.matmul(
                        warm[:], gt[:], gt[:], start=True, stop=True
                    )

                # ---- Z + V projections ----
                # Z first, chunks descending (the first attention unit is
                # qi=7 and needs only Z chunk 7 + its replication, so
                # attention can start ~10us in). Z runs as K=64 2-way
                # row-tiled pairs (128 active PE rows -- flips the HAM
                # clock gate to 2.4 GHz before attention). G's bias column
                # rides in as a per-partition bias add on the PSUM->SBUF
                # copy. V after Z (cheap, off the critical path; low key
                # blocks first since the first unit's PV consumes those).
                for c in reversed(range(_S // 512)):
                    csl = slice(512 * c, 512 * (c + 1))
                    ps = ps_sc.tile([128, 512 * _CHUNK], f32, tag="sc", name="ps")
                    for p in range(2):
                        g = 64 * p
                        xsrc = xT if p == 0 else xTc
                        nc.tensor.matmul(
                            ps[0:_D, 512 * p : 512 * p + 512],
                            gt[g : g + 64, g : g + 64],
                            xsrc[g : g + 64, csl],
                            start=True,
                            stop=True,
                            tile_position=(g, 0),
                        )
                    for p in range(2):
                        pz = ps[0:_D, 512 * p : 512 * p + 512]
                        if (2 * c + p) % 2 == 0:
                            nc.vector.tensor_scalar(
                                Zt[p][0:64, csl], pz, gb[64 * p : 64 * p + 64, 0:1],
                                None, Add,
                            )
                        else:
                            nc.scalar.activation(
                                out=Zt[p][0:64, csl], in_=pz,
                                func=mybir.ActivationFunctionType.Identity,
                                bias=gb[64 * p : 64 * p + 64, 0:1],
                            )
                    for p in range(2):
                        nc.gpsimd.dma_start(
                            out=Zt[p][64:128, csl],
                            in_=Zt[p][0:64, csl],
                        )
                for half in range(2):
                    pv = ps_o.tile([128, 512], f32, tag="po", name="po")
                    for s2 in range(16):
                        s = 16 * half + s2
                        nc.tensor.matmul(
                            pv[:, 32 * s2 : 32 * (s2 + 1)],
                            xT[:, 128 * s : 128 * (s + 1)],
                            wv[:],
                            start=True,
                            stop=True,
                        )
                    vsrc = pv[:].rearrange("p (f c) -> p f c", f=16)
                    for vp in range(2):
                        dst = V[vp][
                            :, 17 * 16 * half : 17 * 16 * (half + 1)
                        ].rearrange("p (f c) -> p f c", f=16)[:, :, 0:16]
                        if vp == 0:
                            nc.vector.tensor_copy(dst, vsrc[:, :, 0:16])
                        else:
                            nc.scalar.copy(dst, vsrc[:, :, 16:32])

                # ---- attention ----
                def emit_score_chunk(p, qi, pt, b0):
                    """One chunk of 2-way row-tiled K=64 score matmuls +
                    mask + alternating-engine exp.

                    qi<2 (queries 0..1023, short concentrated softmax sums)
                    always get exact ACT exp; everything else alternates
                    DVE Schraudolph / ACT exact so consecutive chunks'
                    exps overlap on the two engines. Masked scores on the
                    DVE path land at -1.8M after the affine map and
                    saturate the int16 conversion to 0x8000 = bf16 -0.0.
                    """
                    nkb = 4 * qi + 4
                    qsl = slice(512 * qi, 512 * (qi + 1))
                    nblk = min(_CHUNK, nkb - b0)
                    ps = ps_sc.tile([128, 512 * _CHUNK], f32, tag="sc", name="ps")
                    j0 = b0 - 4 * qi  # j of first block (diag if >= 0)
                    for t in range(nblk):
                        b = b0 + t
                        g = 64 * (b % 2)
                        xsrc = xT if g == 0 else xTc
                        j = b - 4 * qi
                        qoff = 128 * j if j > 0 else 0  # masked-out prefix
                        nc.tensor.matmul(
                            ps[:, 512 * t + qoff : 512 * (t + 1)],
                            xsrc[g : g + 64, 128 * b : 128 * (b + 1)],
                            Zt[p][g : g + 64, 512 * qi + qoff : 512 * (qi + 1)],
                            start=True,
                            stop=True,
                            tile_position=(g, 0),
                        )
                        if j >= 0:  # diagonal block: causal mask
                            sl = ps[:, 512 * t + 128 * j : 512 * t + 128 * (j + 1)]
                            nc.vector.tensor_add(sl, sl, maskt[:])
                            load["dve"] += 128 * DVE_NS + OVHD
                    if qi < 2:
                        eng = "act"
                        load["act"] += 512 * nblk * ACT_NS + OVH
                    else:
                        eng = pick(512 * nblk * ACT_NS + OVH, 512 * nblk * DVE_NS + OVHD)
                    eoff = 128 * j0 if j0 > 0 else 0  # skip masked prefix
                    if eng == "act":
                        nc.scalar.activation(
                            out=pt[:, 512 * b0 + eoff : 512 * (b0 + nblk)],
                            in_=ps[:, eoff : 512 * nblk],
                            func=Exp,
                        )
                    else:
                        pt_i16 = pt[:, 512 * b0 + eoff : 512 * (b0 + nblk)].bitcast(i16)
                        nc.vector.tensor_scalar(
                            pt_i16,
                            ps[:, eoff : 512 * nblk],
                            _EXP_A2,
                            _EXP_B2,
                            Mult,
                            Add,
                        )

                class BUnit:
                    """PV matmuls (K=128, HAM-busy) + store, emitted
                    incrementally so they interleave with the next unit's
                    score chunks."""

                    def __init__(self, p, qi, pt):
                        self.p, self.qi, self.pt = p, qi, pt
                        self.nkb = 4 * qi + 4
                        self.done = 0
                        self.po = ps_o.tile([17, 512], f32, tag="po", name="po")

                    def emit_upto(self, k):
                        for b in range(self.done, min(k, self.nkb)):
                            j = b - 4 * self.qi
                            qoff = 128 * j if j > 0 else 0
                            nc.tensor.matmul(
                                self.po[:, qoff:512],
                                V[self.p][:, 17 * b : 17 * b + 17],
                                self.pt[:, 512 * b + qoff : 512 * (b + 1)],
                                start=(b == 0),
                                stop=(b == self.nkb - 1),
                            )
                        self.done = max(self.done, min(k, self.nkb))

                    def finish(self):
                        self.emit_upto(self.nkb)
                        qsl = slice(512 * self.qi, 512 * (self.qi + 1))
                        ost = stg.tile([17, 512], f32, tag="ost", name="ost")
                        eng = pick(512 * ACT_NS + OVH, 512 * DVE_NS + OVHD)
                        if eng == "act":
                            nc.scalar.copy(ost[:], self.po[:])
                        else:
                            nc.vector.tensor_copy(ost[:], self.po[:])
                        nc.sync.dma_start(out=out_d[self.p][:, qsl], in_=ost[:])

                # Fine-grained software pipeline: while emitting unit i's
                # score chunks (paced by the exp engines), interleave unit
                # i-1's PV matmuls proportionally so the PE never idles.
                # big units first (pipeline ramp), tiny all-exact units
                # (qi 0/1, ACT-serial exp) tucked mid-schedule where the
                # neighbours' PV keeps the PE fed, medium unit last so the
                # drain tail is short
                units = [(p, qi) for qi in (7, 6, 5, 4, 0, 1, 3, 2) for p in range(2)]
                prev = None
                for ui, (p, qi) in enumerate(units):
                    nkb = 4 * qi + 4
                    nchunks = (nkb + _CHUNK - 1) // _CHUNK
                    pt = ptp.tile([128, 512 * _NKB], bf, tag="pt", name="pt")
                    unit = BUnit(p, qi, pt)
                    selfpace = prev is None or ui == len(units) - 1
                    for c in range(nchunks):
                        # PV batch first: gives the PE ready work while the
                        # exp of the previous chunks drains the PSUM pool
                        if prev is not None:
                            prev.emit_upto(((c + 1) * prev.nkb) // nchunks)
                        emit_score_chunk(p, qi, pt, c * _CHUNK)
                        if selfpace and c >= 1:
                            # first unit: self-paced PV (1-chunk lag) gives
                            # the PE K=128 work from the start (HAM warm-up);
                            # last unit: self-paced PV shortens the drain
                            unit.emit_upto(c * _CHUNK)
                        if prev is None:
                            # K=128 filler in otherwise-idle exp-wait slots
                            # so the HAM activity window never reads idle
                            # during the pipeline ramp
                            for _ in range(2):
                                nc.tensor.matmul(
                                    warm[:], gt[:], gt[:], start=True, stop=True
                                )
                    if prev is not None:
                        prev.finish()
                    prev = unit
                prev.finish()

    nc.compile()
    return nc


def _get_nc():
    if "nc" not in _cache:
        _cache["nc"] = _build_nc()
    return _cache["nc"]


def _prepare_in_maps(x, Wq, bq, Wk, bk, Wv, bv):
    bf = ml_dtypes.bfloat16
    x = np.asarray(x, np.float32)
    Wq, bq = np.asarray(Wq, np.float32), np.asarray(bq, np.float32)
    Wk, bk = np.asarray(Wk, np.float32), np.asarray(bk, np.float32)
    Wv, bv = np.asarray(Wv, np.float32), np.asarray(bv, np.float32)
    ones = np.ones((1, _S), np.float32)

    def aug(W, b, h, scale=1.0):
        # [Dh, D+1] block for head h: weight rows plus bias column
        blk = np.concatenate(
            [W[h * _Dh : (h + 1) * _Dh, :], b[h * _Dh : (h + 1) * _Dh, None]], axis=1
        )
        return (blk * scale).T.astype(np.float32)  # [D+1, Dh]

    mask = np.where(
        np.arange(128)[:, None] > np.arange(128)[None, :], _MASK_NEG, 0.0
    ).astype(np.float32)

    in_maps = []
    for c in range(_NC):
        b_idx = c // 2
        heads = (2 * (c % 2), 2 * (c % 2) + 1)
        xT = np.concatenate([x[b_idx].T, ones], axis=0)  # [65, 4096]
        gtm = np.zeros((128, 128), np.float32)
        gbv = np.zeros((128, 1), np.float32)
        wv_cols = []
        for p, h in enumerate(heads):
            Wq_aug = aug(Wq, bq, h, _SCALE)  # [65, 16], q-scaled
            Wk_aug = aug(Wk, bk, h)  # [65, 16]
            G = Wk_aug @ Wq_aug.T  # [65, 65] = Wk_aug^T Wq_aug in row form
            gtm[64 * p : 64 * p + 64, 64 * p : 64 * p + 64] = G[0:64, 0:64].T
            gbv[64 * p : 64 * p + 64, 0] = G[0:64, 64]
            wv_cols.append(aug(Wv, bv, h))
        in_maps.append(
            {
                "xT": xT.astype(bf),
                "gt": gtm.astype(bf),
                "gb": gbv,
                "wv": np.concatenate(wv_cols, axis=1).astype(bf),
                "mask": mask,
            }
        )
    return in_maps


def _assemble(results):
    final = np.empty((_B, _S, _D), np.float32)
    for c in range(_NC):
        b_idx = c // 2
        for p in range(2):
            h = 2 * (c % 2) + p
            o = np.asarray(results[c]["out"], np.float32)  # [2, 17, S]
            final[b_idx, :, h * _Dh : (h + 1) * _Dh] = (o[p, :16] / o[p, 16:17]).T
    return final


def _run(in_maps, trace=False, trace_kwargs=None):
    from concourse.bass_utils import run_bass_kernel_spmd

    nc = _get_nc()
    return run_bass_kernel_spmd(
        nc, in_maps, list(range(_NC)), trace=trace, **(trace_kwargs or {})
    )


def kernel(x, Wq, bq, Wk, bk, Wv, bv):
    in_maps = _prepare_in_maps(x, Wq, bq, Wk, bk, Wv, bv)
    res = _run(in_maps)
    return _assemble(res.results)


# revision 25
# speedup vs baseline: 1.0525x; 1.0525x over previous
"""Causal self-attention (B=4, S=4096, D=64, H=4) on 8 TRN2 NeuronCores.

Sharding: the 16 (batch, head) pairs are distributed 2-per-core
(core c -> batch c//2, heads (2*(c%2), 2*(c%2)+1)). Each core runs the
full fused attention for its 2 pairs; no cross-core communication.

Two tricks carry the kernel:

1. BILINEAR SCORES (K=64 keeps the PE's HAM clock gate warm). The
   TRN2 PE clock-gates to 1.2 GHz unless the activity monitor sees
   high-K matmuls (measured: K=16 4-way row-tiled matmuls never warm
   the clock, K>=64 does). Instead of Q@K^T with its K=16 (head dim)
   contraction, fold the projections into a host-precomputed bilinear
   form G_h = Wk_aug^T Wq_aug / sqrt(Dh) [65,65]:
       scores^T = x_aug G x_aug^T = x @ (G[0:64,:] @ x_aug^T)
   The per-query row G[64,:]@... cancels in softmax and is dropped.
   On device: Z_p = G64_p @ x_aug^T (K=65), then each score block is
   x^T_block @ Z_p with K=64, 2-way row-tiled (tile_position 0/64) --
   every matmul in the kernel now registers as HAM-busy, holding
   2.4 GHz, and the Q/K projection pass disappears entirely.

2. TWO-ENGINE SOFTMAX EXP (breaks the ACT throughput wall: 18.9M
   exps/core at 1 elem/cycle/lane is ~123us on ACT alone). Diagonal
   (causal-masked) key blocks take exact ACT exp; off-diagonal blocks
   are greedily balanced between ACT exact exp and a one-instruction
   DVE Schraudolph exp emitting bf16 BITS directly:
       bf16_bits(exp(s)) ~= int16(184.665*s + B2)
   (tensor_scalar mult+add, f32 PSUM in -> int16 SBUF out, bitcast
   back to bf16 for the P@V matmul). Keeping diagonal blocks exact
   confines the ~3% Schraudolph weight error to wide softmax sums
   where it averages out (measured rel err 0.011, tolerance 2e-2)
   and keeps -1e4 masked scores away from int16 saturation.

Scores are computed TRANSPOSED (key position on partitions) so P@V
needs no transpose; the softmax denominator comes free from a 17th
all-ones column in V; the final division happens on host. Exp skips
max-subtraction (scores are O(13), f32 exp cannot overflow).
"""

import numpy as np
import ml_dtypes

_B, _S, _D = 4, 4096, 64
_H, _Dh = 4, 16
_NC = 8
_SCALE = 1.0 / np.sqrt(_Dh)
_MASK_NEG = -10000.0
_NQB = _S // 512  # 8 query super-blocks of 512
_NKB = _S // 128  # 32 key blocks of 128
_CHUNK = 2  # k-blocks per exp chunk (2 PSUM banks, 3 bufs)

# Schraudolph exp -> bf16 bits: i16 = A2*s + B2, bitcast to bf16
_EXP_A2 = (2.0**23 / np.log(2.0)) / 65536.0
_EXP_B2 = (127.0 * 2.0**23 - 366393.0) / 65536.0

_cache = {}


def _build_nc():
    import concourse.tile as tile
    from concourse import bacc, mybir

    bf = mybir.dt.bfloat16
    i16 = mybir.dt.int16
    f32 = mybir.dt.float32
    Exp = mybir.ActivationFunctionType.Exp
    Mult = mybir.AluOpType.mult
    Add = mybir.AluOpType.add

    nc = bacc.Bacc("TRN2", target_bir_lowering=False, debug=False, num_devices=_NC)
    xT_d = nc.dram_tensor("xT", [_D + 1, _S], bf, kind="ExternalInput").ap()
    gt_d = nc.dram_tensor("gt", [128, 128], bf, kind="ExternalInput").ap()
    gb_d = nc.dram_tensor("gb", [128, 1], f32, kind="ExternalInput").ap()
    wv_d = nc.dram_tensor("wv", [_D + 1, 32], bf, kind="ExternalInput").ap()
    mask_d = nc.dram_tensor("mask", [128, 128], f32, kind="ExternalInput").ap()
    out_d = nc.dram_tensor("out", [2, 17, _S], f32, kind="ExternalOutput").ap()

    # greedy ACT/DVE balance: estimated busy ns accumulated per engine
    load = {"act": 0.0, "dve": 0.0, "n": 0}
    ACT_NS, DVE_NS = 1e9 / 1.2e9, 1e9 / 0.96e9  # per lane-element
    OVH = 350.0  # ACT per-instruction overhead estimate
    OVHD = 230.0  # DVE per-instruction overhead estimate

    def pick(act_cost, dve_cost):
        if load["act"] + act_cost <= load["dve"] + dve_cost:
            load["act"] += act_cost
            return "act"
        load["dve"] += dve_cost
        return "dve"

    with tile.TileContext(nc) as tc:
        with tc.tile_pool(name="singles", bufs=1) as singles:
            xT = singles.tile([_D + 1, _S], bf, tag="xT")
            xTc = singles.tile([128, _S], bf, tag="xTc")
            gt = singles.tile([128, 128], bf, tag="gt")
            gb = singles.tile([128, 1], f32, tag="gb")
            wv = singles.tile([_D + 1, 32], bf, tag="wv")
            maskt = singles.tile([128, 128], f32, tag="mask")
            # gt/wv first (tiny, unblock Z/V matmuls), xT split across the
            # sync and scalar HWDGE queues (each partition row is one DMA
            # descriptor, so a single transfer serializes ~65 of them);
            # odd-row score copy of x^T replicated SBUF->SBUF on the
            # gpsimd DGE as each chunk lands.
            nc.sync.dma_start(out=gt[:], in_=gt_d)
            nc.sync.dma_start(out=gb[:], in_=gb_d)
            nc.scalar.dma_start(out=wv[:], in_=wv_d)
            # high xT columns first: the first attention unit is qi=7, so
            # the projection consumes chunks in descending order
            for i, c in enumerate((3, 2, 1, 0)):
                eng = nc.sync if i % 2 == 0 else nc.scalar
                eng.dma_start(
                    out=xT[:, 1024 * c : 1024 * (c + 1)],
                    in_=xT_d[:, 1024 * c : 1024 * (c + 1)],
                )
                nc.gpsimd.dma_start(
                    out=xTc[64:128, 1024 * c : 1024 * (c + 1)],
                    in_=xT[0:64, 1024 * c : 1024 * (c + 1)],
                )
            nc.scalar.dma_start(out=maskt[:], in_=mask_d)

            # Z_p replicated at partition offsets 0/64 for 2-way row-tiled
            # score matmuls (xTc comes replicated from host).
            Zt = [singles.tile([128, _S], bf, tag=f"Zt{p}", name=f"Zt{p}") for p in range(2)]
            V = [singles.tile([128, 17 * _NKB], bf, tag=f"V{p}", name=f"V{p}") for p in range(2)]
            for p in range(2):
                nc.vector.memset(V[p][:], 1.0)

            # M2 variants for the fused DVE diag exp: (ps*A2) + M2 with
            # M2 = B2 + A2*mask laid out per diag-chunk shape. Variant h
            # has mask squares at chunk cols [256h, 256h+128) (slot 0,
            # block j=2h) and [512+256h+128, +128) (slot 1, block j=2h+1).
            M2 = singles.tile([128, 2048], f32, tag="M2", name="M2")
            nc.vector.memset(M2[:], _EXP_B2)
            for h in range(2):
                for t in range(2):
                    sq = slice(2048 * 0 + 1024 * h + 512 * t + 128 * (2 * h + t), 1024 * h + 512 * t + 128 * (2 * h + t) + 128)
                    nc.vector.tensor_scalar(
                        M2[:, sq], maskt[:], _EXP_A2, _EXP_B2, Mult, Add
                    )

            # ---- attention pools (opened early: the Z/V projection
            # draws its PSUM tiles from the same pools, so there is no
            # pool-boundary barrier between projection and attention) ----
            with (
                tc.tile_pool(name="ps_sc", bufs=3, space="PSUM") as ps_sc,
                tc.tile_pool(name="ps_o", bufs=2, space="PSUM") as ps_o,
                tc.tile_pool(name="ptp", bufs=2) as ptp,
                tc.tile_pool(name="stg", bufs=3) as stg,
            ):
                # ---- HAM warm-up burst ----
                # ~4.5us of dense K=128 matmuls on the (tiny, first-to-
                # arrive) gt tile while the PE would otherwise idle waiting
                # for the xT DMA: flips the clock gate to 2.4 GHz before
                # the real work starts. Output is never read.
                warm = ps_o.tile([128, 128], f32, tag="po", name="po")
                for _ in range(44):
                    nc.tensor.matmul(
                        warm[:], gt[:], gt[:], start=True, stop=True
                    )

                # ---- Z + V projections ----
                # Z first, chunks descending (the first attention unit is
                # qi=7 and needs only Z chunk 7 + its replication, so
                # attention can start ~10us in). Z runs as K=64 2-way
                # row-tiled pairs (128 active PE rows -- flips the HAM
                # clock gate to 2.4 GHz before attention). G's bias column
                # rides in as a per-partition bias add on the PSUM->SBUF
                # copy. V after Z (cheap, off the critical path; low key
                # blocks first since the first unit's PV consumes those).
                for c in reversed(range(_S // 512)):
                    csl = slice(512 * c, 512 * (c + 1))
                    ps = ps_sc.tile([128, 512 * _CHUNK], f32, tag="sc", name="ps")
                    for p in range(2):
                        g = 64 * p
                        xsrc = xT if p == 0 else xTc
                        nc.tensor.matmul(
                            ps[0:_D, 512 * p : 512 * p + 512],
                            gt[g : g + 64, g : g + 64],
                            xsrc[g : g + 64, csl],
                            start=True,
                            stop=True,
                            tile_position=(g, 0),
                        )
                    for p in range(2):
                        pz = ps[0:_D, 512 * p : 512 * p + 512]
                        if (2 * c + p) % 2 == 0:
                            nc.vector.tensor_scalar(
                                Zt[p][0:64, csl], pz, gb[64 * p : 64 * p + 64, 0:1],
                                None, Add,
                            )
                        else:
                            nc.scalar.activation(
                                out=Zt[p][0:64, csl], in_=pz,
                                func=mybir.ActivationFunctionType.Identity,
                                bias=gb[64 * p : 64 * p + 64, 0:1],
                            )
                    for p in range(2):
                        nc.gpsimd.dma_start(
                            out=Zt[p][64:128, csl],
                            in_=Zt[p][0:64, csl],
                        )
                for half in range(2):
                    pv = ps_o.tile([128, 512], f32, tag="po", name="po")
                    for s2 in range(16):
                        s = 16 * half + s2
                        nc.tensor.matmul(
                            pv[:, 32 * s2 : 32 * (s2 + 1)],
                            xT[:, 128 * s : 128 * (s + 1)],
                            wv[:],
                            start=True,
                            stop=True,
                        )
                    vsrc = pv[:].rearrange("p (f c) -> p f c", f=16)
                    for vp in range(2):
                        dst = V[vp][
                            :, 17 * 16 * half : 17 * 16 * (half + 1)
                        ].rearrange("p (f c) -> p f c", f=16)[:, :, 0:16]
                        if vp == 0:
                            nc.vector.tensor_copy(dst, vsrc[:, :, 0:16])
                        else:
                            nc.scalar.copy(dst, vsrc[:, :, 16:32])

                # ---- attention ----
                def emit_score_chunk(p, qi, pt, b0):
                    """One chunk of 2-way row-tiled K=64 score matmuls +
                    mask + alternating-engine exp.

                    qi<2 (queries 0..1023, short concentrated softmax sums)
                    always get exact ACT exp; everything else alternates
                    DVE Schraudolph / ACT exact so consecutive chunks'
                    exps overlap on the two engines. Masked scores on the
                    DVE path land at -1.8M after the affine map and
                    saturate the int16 conversion to 0x8000 = bf16 -0.0.
                    """
                    nkb = 4 * qi + 4
                    qsl = slice(512 * qi, 512 * (qi + 1))
                    nblk = min(_CHUNK, nkb - b0)
                    ps = ps_sc.tile([128, 512 * _CHUNK], f32, tag="sc", name="ps")
                    j0 = b0 - 4 * qi  # j of first block (diag if >= 0)
                    for t in range(nblk):
                        b = b0 + t
                        g = 64 * (b % 2)
                        xsrc = xT if g == 0 else xTc
                        j = b - 4 * qi
                        qoff = 128 * j if j > 0 else 0  # masked-out prefix
                        nc.tensor.matmul(
                            ps[:, 512 * t + qoff : 512 * (t + 1)],
                            xsrc[g : g + 64, 128 * b : 128 * (b + 1)],
                            Zt[p][g : g + 64, 512 * qi + qoff : 512 * (qi + 1)],
                            start=True,
                            stop=True,
                            tile_position=(g, 0),
                        )
                        if j >= 0 and qi < 2:
                            # diagonal block causal mask; for qi>=2 the
                            # mask is fused into the DVE exp via M2
                            sl = ps[:, 512 * t + 128 * j : 512 * t + 128 * (j + 1)]
                            nc.vector.tensor_add(sl, sl, maskt[:])
                            load["dve"] += 128 * DVE_NS + OVHD
                    if qi < 2:
                        eng = "act"
                        load["act"] += 512 * nblk * ACT_NS + OVH
                    elif j0 >= 0:
                        eng = "dve_fused"  # diag chunk: mask fused into exp
                        load["dve"] += 512 * nblk * DVE_NS + OVHD
                    else:
                        eng = pick(512 * nblk * ACT_NS + OVH, 512 * nblk * DVE_NS + OVHD)
                    eoff = 128 * j0 if j0 > 0 else 0  # skip masked prefix
                    if eng == "dve_fused":
                        h = j0 // 2
                        pt_i16 = pt[:, 512 * b0 + eoff : 512 * (b0 + nblk)].bitcast(i16)
                        nc.vector.scalar_tensor_tensor(
                            pt_i16,
                            ps[:, eoff : 512 * nblk],
                            _EXP_A2,
                            M2[:, 1024 * h + eoff : 1024 * h + 512 * nblk],
                            Mult,
                            Add,
                        )
                    elif eng == "act":
                        nc.scalar.activation(
                            out=pt[:, 512 * b0 + eoff : 512 * (b0 + nblk)],
                            in_=ps[:, eoff : 512 * nblk],
                            func=Exp,
                        )
                    else:
                        pt_i16 = pt[:, 512 * b0 + eoff : 512 * (b0 + nblk)].bitcast(i16)
                        nc.vector.tensor_scalar(
                            pt_i16,
                            ps[:, eoff : 512 * nblk],
                            _EXP_A2,
                            _EXP_B2,
                            Mult,
                            Add,
                        )

                class BUnit:
                    """PV matmuls (K=128, HAM-busy) + store, emitted
                    incrementally so they interleave with the next unit's
                    score chunks."""

                    def __init__(self, p, qi, pt):
                        self.p, self.qi, self.pt = p, qi, pt
                        self.nkb = 4 * qi + 4
                        self.done = 0
                        self.po = ps_o.tile([17, 512], f32, tag="po", name="po")

                    def emit_upto(self, k):
                        for b in range(self.done, min(k, self.nkb)):
                            j = b - 4 * self.qi
                            qoff = 128 * j if j > 0 else 0
                            nc.tensor.matmul(
                                self.po[:, qoff:512],
                                V[self.p][:, 17 * b : 17 * b + 17],
                                self.pt[:, 512 * b + qoff : 512 * (b + 1)],
                                start=(b == 0),
                                stop=(b == self.nkb - 1),
                            )
                        self.done = max(self.done, min(k, self.nkb))

                    def finish(self):
                        self.emit_upto(self.nkb)
                        qsl = slice(512 * self.qi, 512 * (self.qi + 1))
                        ost = stg.tile([17, 512], f32, tag="ost", name="ost")
                        eng = pick(512 * ACT_NS + OVH, 512 * DVE_NS + OVHD)
                        if eng == "act":
                            nc.scalar.copy(ost[:], self.po[:])
                        else:
                            nc.vector.tensor_copy(ost[:], self.po[:])
                        nc.sync.dma_start(out=out_d[self.p][:, qsl], in_=ost[:])

                # Fine-grained software pipeline: while emitting unit i's
                # score chunks (paced by the exp engines), interleave unit
                # i-1's PV matmuls proportionally so the PE never idles.
                # big units first (pipeline ramp), tiny all-exact units
                # (qi 0/1, ACT-serial exp) tucked mid-schedule where the
                # neighbours' PV keeps the PE fed, medium unit last so the
                # drain tail is short
                units = [(p, qi) for qi in (7, 6, 5, 4, 0, 1, 3, 2) for p in range(2)]
                prev = None
                for ui, (p, qi) in enumerate(units):
                    nkb = 4 * qi + 4
                    nchunks = (nkb + _CHUNK - 1) // _CHUNK
                    pt = ptp.tile([128, 512 * _NKB], bf, tag="pt", name="pt")
                    unit = BUnit(p, qi, pt)
                    selfpace = prev is None or ui == len(units) - 1
                    for c in range(nchunks):
                        # PV batch first: gives the PE ready work while the
                        # exp of the previous chunks drains the PSUM pool
                        if prev is not None:
                            prev.emit_upto(((c + 1) * prev.nkb) // nchunks)
                        emit_score_chunk(p, qi, pt, c * _CHUNK)
                        if selfpace and c >= 1:
                            # first unit: self-paced PV (1-chunk lag) gives
                            # the PE K=128 work from the start (HAM warm-up);
                            # last unit: self-paced PV shortens the drain
                            unit.emit_upto(c * _CHUNK)
                        if prev is None:
                            # K=128 filler in otherwise-idle exp-wait slots
                            # so the HAM activity window never reads idle
                            # during the pipeline ramp
                            for _ in range(2):
                                nc.tensor.matmul(
                                    warm[:], gt[:], gt[:], start=True, stop=True
                                )
                    if prev is not None:
                        prev.finish()
                    prev = unit
                prev.finish()

    nc.compile()
    return nc


def _get_nc():
    if "nc" not in _cache:
        _cache["nc"] = _build_nc()
    return _cache["nc"]


def _prepare_in_maps(x, Wq, bq, Wk, bk, Wv, bv):
    bf = ml_dtypes.bfloat16
    x = np.asarray(x, np.float32)
    Wq, bq = np.asarray(Wq, np.float32), np.asarray(bq, np.float32)
    Wk, bk = np.asarray(Wk, np.float32), np.asarray(bk, np.float32)
    Wv, bv = np.asarray(Wv, np.float32), np.asarray(bv, np.float32)
    ones = np.ones((1, _S), np.float32)

    def aug(W, b, h, scale=1.0):
        # [Dh, D+1] block for head h: weight rows plus bias column
        blk = np.concatenate(
            [W[h * _Dh : (h + 1) * _Dh, :], b[h * _Dh : (h + 1) * _Dh, None]], axis=1
        )
        return (blk * scale).T.astype(np.float32)  # [D+1, Dh]

    mask = np.where(
        np.arange(128)[:, None] > np.arange(128)[None, :], _MASK_NEG, 0.0
    ).astype(np.float32)

    in_maps = []
    for c in range(_NC):
        b_idx = c // 2
        heads = (2 * (c % 2), 2 * (c % 2) + 1)
        xT = np.concatenate([x[b_idx].T, ones], axis=0)  # [65, 4096]
        gtm = np.zeros((128, 128), np.float32)
        gbv = np.zeros((128, 1), np.float32)
        wv_cols = []
        for p, h in enumerate(heads):
            Wq_aug = aug(Wq, bq, h, _SCALE)  # [65, 16], q-scaled
            Wk_aug = aug(Wk, bk, h)  # [65, 16]
            G = Wk_aug @ Wq_aug.T  # [65, 65] = Wk_aug^T Wq_aug in row form
            gtm[64 * p : 64 * p + 64, 64 * p : 64 * p + 64] = G[0:64, 0:64].T
            gbv[64 * p : 64 * p + 64, 0] = G[0:64, 64]
            wv_cols.append(aug(Wv, bv, h))
        in_maps.append(
            {
                "xT": xT.astype(bf),
                "gt": gtm.astype(bf),
                "gb": gbv,
                "wv": np.concatenate(wv_cols, axis=1).astype(bf),
                "mask": mask,
            }
        )
    return in_maps


def _assemble(results):
    final = np.empty((_B, _S, _D), np.float32)
    for c in range(_NC):
        b_idx = c // 2
        for p in range(2):
            h = 2 * (c % 2) + p
            o = np.asarray(results[c]["out"], np.float32)  # [2, 17, S]
            final[b_idx, :, h * _Dh : (h + 1) * _Dh] = (o[p, :16] / o[p, 16:17]).T
    return final


def _run(in_maps, trace=False, trace_kwargs=None):
    from concourse.bass_utils import run_bass_kernel_spmd

    nc = _get_nc()
    return run_bass_kernel_spmd(
        nc, in_maps, list(range(_NC)), trace=trace, **(trace_kwargs or {})
    )


def kernel(x, Wq, bq, Wk, bk, Wv, bv):
    in_maps = _prepare_in_maps(x, Wq, bq, Wk, bk, Wv, bv)
    res = _run(in_maps)
    return _assemble(res.results)


# revision 26
# speedup vs baseline: 1.0650x; 1.0119x over previous
"""Causal self-attention (B=4, S=4096, D=64, H=4) on 8 TRN2 NeuronCores.

Sharding: the 16 (batch, head) pairs are distributed 2-per-core
(core c -> batch c//2, heads (2*(c%2), 2*(c%2)+1)). Each core runs the
full fused attention for its 2 pairs; no cross-core communication.

Two tricks carry the kernel:

1. BILINEAR SCORES (K=64 keeps the PE's HAM clock gate warm). The
   TRN2 PE clock-gates to 1.2 GHz unless the activity monitor sees
   high-K matmuls (measured: K=16 4-way row-tiled matmuls never warm
   the clock, K>=64 does). Instead of Q@K^T with its K=16 (head dim)
   contraction, fold the projections into a host-precomputed bilinear
   form G_h = Wk_aug^T Wq_aug / sqrt(Dh) [65,65]:
       scores^T = x_aug G x_aug^T = x @ (G[0:64,:] @ x_aug^T)
   The per-query row G[64,:]@... cancels in softmax and is dropped.
   On device: Z_p = G64_p @ x_aug^T (K=65), then each score block is
   x^T_block @ Z_p with K=64, 2-way row-tiled (tile_position 0/64) --
   every matmul in the kernel now registers as HAM-busy, holding
   2.4 GHz, and the Q/K projection pass disappears entirely.

2. TWO-ENGINE SOFTMAX EXP (breaks the ACT throughput wall: 18.9M
   exps/core at 1 elem/cycle/lane is ~123us on ACT alone). Diagonal
   (causal-masked) key blocks take exact ACT exp; off-diagonal blocks
   are greedily balanced between ACT exact exp and a one-instruction
   DVE Schraudolph exp emitting bf16 BITS directly:
       bf16_bits(exp(s)) ~= int16(184.665*s + B2)
   (tensor_scalar mult+add, f32 PSUM in -> int16 SBUF out, bitcast
   back to bf16 for the P@V matmul). Keeping diagonal blocks exact
   confines the ~3% Schraudolph weight error to wide softmax sums
   where it averages out (measured rel err 0.011, tolerance 2e-2)
   and keeps -1e4 masked scores away from int16 saturation.

Scores are computed TRANSPOSED (key position on partitions) so P@V
needs no transpose; the softmax denominator comes free from a 17th
all-ones column in V; the final division happens on host. Exp skips
max-subtraction (scores are O(13), f32 exp cannot overflow).
"""

import numpy as np
import ml_dtypes

_B, _S, _D = 4, 4096, 64
_H, _Dh = 4, 16
_NC = 8
_SCALE = 1.0 / np.sqrt(_Dh)
_MASK_NEG = -10000.0
_NQB = _S // 512  # 8 query super-blocks of 512
_NKB = _S // 128  # 32 key blocks of 128
_CHUNK = 2  # k-blocks per exp chunk (2 PSUM banks, 3 bufs)

# Schraudolph exp -> bf16 bits: i16 = A2*s + B2, bitcast to bf16
_EXP_A2 = (2.0**23 / np.log(2.0)) / 65536.0
_EXP_B2 = (127.0 * 2.0**23 - 366393.0) / 65536.0

_cache = {}


def _build_nc():
    import concourse.tile as tile
    from concourse import bacc, mybir

    bf = mybir.dt.bfloat16
    i16 = mybir.dt.int16
    f32 = mybir.dt.float32
    Exp = mybir.ActivationFunctionType.Exp
    Mult = mybir.AluOpType.mult
    Add = mybir.AluOpType.add

    nc = bacc.Bacc("TRN2", target_bir_lowering=False, debug=False, num_devices=_NC)
    xT_d = nc.dram_tensor("xT", [_D + 1, _S], bf, kind="ExternalInput").ap()
    gt_d = nc.dram_tensor("gt", [128, 128], bf, kind="ExternalInput").ap()
    gb_d = nc.dram_tensor("gb", [128, 1], f32, kind="ExternalInput").ap()
    wv_d = nc.dram_tensor("wv", [_D + 1, 32], bf, kind="ExternalInput").ap()
    mask_d = nc.dram_tensor("mask", [128, 128], f32, kind="ExternalInput").ap()
    out_d = nc.dram_tensor("out", [2, 17, _S], f32, kind="ExternalOutput").ap()

    # greedy ACT/DVE balance: estimated busy ns accumulated per engine
    load = {"act": 0.0, "dve": 0.0, "n": 0}
    ACT_NS, DVE_NS = 1e9 / 1.2e9, 1e9 / 0.96e9  # per lane-element
    OVH = 350.0  # ACT per-instruction overhead estimate
    OVHD = 230.0  # DVE per-instruction overhead estimate

    def pick(act_cost, dve_cost):
        if load["act"] + act_cost <= load["dve"] + dve_cost:
            load["act"] += act_cost
            return "act"
        load["dve"] += dve_cost
        return "dve"

    with tile.TileContext(nc) as tc:
        with tc.tile_pool(name="singles", bufs=1) as singles:
            xT = singles.tile([_D + 1, _S], bf, tag="xT")
            xTc = singles.tile([128, _S], bf, tag="xTc")
            gt = singles.tile([128, 128], bf, tag="gt")
            gb = singles.tile([128, 1], f32, tag="gb")
            wv = singles.tile([_D + 1, 32], bf, tag="wv")
            maskt = singles.tile([128, 128], f32, tag="mask")
            # gt/wv first (tiny, unblock Z/V matmuls), xT split across the
            # sync and scalar HWDGE queues (each partition row is one DMA
            # descriptor, so a single transfer serializes ~65 of them);
            # odd-row score copy of x^T replicated SBUF->SBUF on the
            # gpsimd DGE as each chunk lands.
            nc.sync.dma_start(out=gt[:], in_=gt_d)
            nc.sync.dma_start(out=gb[:], in_=gb_d)
            nc.scalar.dma_start(out=wv[:], in_=wv_d)
            # high xT columns first: the first attention unit is qi=7, so
            # the projection consumes chunks in descending order
            for i, c in enumerate((3, 2, 1, 0)):
                eng = nc.sync if i % 2 == 0 else nc.scalar
                eng.dma_start(
                    out=xT[:, 1024 * c : 1024 * (c + 1)],
                    in_=xT_d[:, 1024 * c : 1024 * (c + 1)],
                )
                nc.gpsimd.dma_start(
                    out=xTc[64:128, 1024 * c : 1024 * (c + 1)],
                    in_=xT[0:64, 1024 * c : 1024 * (c + 1)],
                )
            nc.scalar.dma_start(out=maskt[:], in_=mask_d)

            # Z_p replicated at partition offsets 0/64 for 2-way row-tiled
            # score matmuls (xTc comes replicated from host).
            Zt = [singles.tile([128, _S], bf, tag=f"Zt{p}", name=f"Zt{p}") for p in range(2)]
            V = [singles.tile([128, 17 * _NKB], bf, tag=f"V{p}", name=f"V{p}") for p in range(2)]
            for p in range(2):
                nc.vector.memset(V[p][:], 1.0)

            # M2 variants for the fused DVE diag exp: (ps*A2) + M2 with
            # M2 = B2 + A2*mask laid out per diag-chunk shape. Variant h
            # has mask squares at chunk cols [256h, 256h+128) (slot 0,
            # block j=2h) and [512+256h+128, +128) (slot 1, block j=2h+1).
            M2 = singles.tile([128, 2048], f32, tag="M2", name="M2")
            nc.vector.memset(M2[:], _EXP_B2)
            for h in range(2):
                for t in range(2):
                    sq = slice(2048 * 0 + 1024 * h + 512 * t + 128 * (2 * h + t), 1024 * h + 512 * t + 128 * (2 * h + t) + 128)
                    nc.vector.tensor_scalar(
                        M2[:, sq], maskt[:], _EXP_A2, _EXP_B2, Mult, Add
                    )

            # ---- attention pools (opened early: the Z/V projection
            # draws its PSUM tiles from the same pools, so there is no
            # pool-boundary barrier between projection and attention) ----
            with (
                tc.tile_pool(name="ps_sc", bufs=3, space="PSUM") as ps_sc,
                tc.tile_pool(name="ps_o", bufs=2, space="PSUM") as ps_o,
                tc.tile_pool(name="ptp", bufs=3) as ptp,
                tc.tile_pool(name="stg", bufs=3) as stg,
            ):
                # ---- HAM warm-up burst ----
                # ~4.5us of dense K=128 matmuls on the (tiny, first-to-
                # arrive) gt tile while the PE would otherwise idle waiting
                # for the xT DMA: flips the clock gate to 2.4 GHz before
                # the real work starts. Output is never read.
                warm = ps_o.tile([32, 128], f32, tag="po", name="po")
                for _ in range(36):
                    nc.tensor.matmul(
                        warm[:], gt[:, 0:32], gt[:], start=True, stop=True
                    )

                # ---- Z + V projections ----
                # Z first, chunks descending (the first attention unit is
                # qi=7 and needs only Z chunk 7 + its replication, so
                # attention can start ~10us in). Z runs as K=64 2-way
                # row-tiled pairs (128 active PE rows -- flips the HAM
                # clock gate to 2.4 GHz before attention). G's bias column
                # rides in as a per-partition bias add on the PSUM->SBUF
                # copy. V after Z (cheap, off the critical path; low key
                # blocks first since the first unit's PV consumes those).
                for c in reversed(range(_S // 512)):
                    csl = slice(512 * c, 512 * (c + 1))
                    ps = ps_sc.tile([128, 512 * _CHUNK], f32, tag="sc", name="ps")
                    for p in range(2):
                        g = 64 * p
                        xsrc = xT if p == 0 else xTc
                        nc.tensor.matmul(
                            ps[0:_D, 512 * p : 512 * p + 512],
                            gt[g : g + 64, g : g + 64],
                            xsrc[g : g + 64, csl],
                            start=True,
                            stop=True,
                            tile_position=(g, 0),
                        )
                    for p in range(2):
                        pz = ps[0:_D, 512 * p : 512 * p + 512]
                        if (2 * c + p) % 2 == 0:
                            nc.vector.tensor_scalar(
                                Zt[p][0:64, csl], pz, gb[64 * p : 64 * p + 64, 0:1],
                                None, Add,
                            )
                        else:
                            nc.scalar.activation(
                                out=Zt[p][0:64, csl], in_=pz,
                                func=mybir.ActivationFunctionType.Identity,
                                bias=gb[64 * p : 64 * p + 64, 0:1],
                            )
                    for p in range(2):
                        nc.gpsimd.dma_start(
                            out=Zt[p][64:128, csl],
                            in_=Zt[p][0:64, csl],
                        )
                for half in range(2):
                    pv = ps_o.tile([128, 512], f32, tag="po", name="po")
                    for s2 in range(16):
                        s = 16 * half + s2
                        nc.tensor.matmul(
                            pv[:, 32 * s2 : 32 * (s2 + 1)],
                            xT[:, 128 * s : 128 * (s + 1)],
                            wv[:],
                            start=True,
                            stop=True,
                        )
                    vsrc = pv[:].rearrange("p (f c) -> p f c", f=16)
                    for vp in range(2):
                        dst = V[vp][
                            :, 17 * 16 * half : 17 * 16 * (half + 1)
                        ].rearrange("p (f c) -> p f c", f=16)[:, :, 0:16]
                        if vp == 0:
                            nc.vector.tensor_copy(dst, vsrc[:, :, 0:16])
                        else:
                            nc.scalar.copy(dst, vsrc[:, :, 16:32])

                # ---- attention ----
                def emit_score_chunk(p, qi, pt, b0):
                    """One chunk of 2-way row-tiled K=64 score matmuls +
                    mask + alternating-engine exp.

                    qi<2 (queries 0..1023, short concentrated softmax sums)
                    always get exact ACT exp; everything else alternates
                    DVE Schraudolph / ACT exact so consecutive chunks'
                    exps overlap on the two engines. Masked scores on the
                    DVE path land at -1.8M after the affine map and
                    saturate the int16 conversion to 0x8000 = bf16 -0.0.
                    """
                    nkb = 4 * qi + 4
                    qsl = slice(512 * qi, 512 * (qi + 1))
                    nblk = min(_CHUNK, nkb - b0)
                    ps = ps_sc.tile([128, 512 * _CHUNK], f32, tag="sc", name="ps")
                    j0 = b0 - 4 * qi  # j of first block (diag if >= 0)
                    for t in range(nblk):
                        b = b0 + t
                        g = 64 * (b % 2)
                        xsrc = xT if g == 0 else xTc
                        j = b - 4 * qi
                        qoff = 128 * j if j > 0 else 0  # masked-out prefix
                        nc.tensor.matmul(
                            ps[:, 512 * t + qoff : 512 * (t + 1)],
                            xsrc[g : g + 64, 128 * b : 128 * (b + 1)],
                            Zt[p][g : g + 64, 512 * qi + qoff : 512 * (qi + 1)],
                            start=True,
                            stop=True,
                            tile_position=(g, 0),
                        )
                        if j >= 0 and qi < 2:
                            # diagonal block causal mask; for qi>=2 the
                            # mask is fused into the DVE exp via M2
                            sl = ps[:, 512 * t + 128 * j : 512 * t + 128 * (j + 1)]
                            nc.vector.tensor_add(sl, sl, maskt[:])
                            load["dve"] += 128 * DVE_NS + OVHD
                    if qi < 2:
                        eng = "act"
                        load["act"] += 512 * nblk * ACT_NS + OVH
                    elif j0 >= 0:
                        eng = "dve_fused"  # diag chunk: mask fused into exp
                        load["dve"] += 512 * nblk * DVE_NS + OVHD
                    else:
                        eng = pick(512 * nblk * ACT_NS + OVH, 512 * nblk * DVE_NS + OVHD)
                    eoff = 128 * j0 if j0 > 0 else 0  # skip masked prefix
                    if eng == "dve_fused":
                        h = j0 // 2
                        pt_i16 = pt[:, 512 * b0 + eoff : 512 * (b0 + nblk)].bitcast(i16)
                        nc.vector.scalar_tensor_tensor(
                            pt_i16,
                            ps[:, eoff : 512 * nblk],
                            _EXP_A2,
                            M2[:, 1024 * h + eoff : 1024 * h + 512 * nblk],
                            Mult,
                            Add,
                        )
                    elif eng == "act":
                        nc.scalar.activation(
                            out=pt[:, 512 * b0 + eoff : 512 * (b0 + nblk)],
                            in_=ps[:, eoff : 512 * nblk],
                            func=Exp,
                        )
                    else:
                        pt_i16 = pt[:, 512 * b0 + eoff : 512 * (b0 + nblk)].bitcast(i16)
                        nc.vector.tensor_scalar(
                            pt_i16,
                            ps[:, eoff : 512 * nblk],
                            _EXP_A2,
                            _EXP_B2,
                            Mult,
                            Add,
                        )

                class BUnit:
                    """PV matmuls (K=128, HAM-busy) + store, emitted
                    incrementally so they interleave with the next unit's
                    score chunks."""

                    def __init__(self, p, qi, pt):
                        self.p, self.qi, self.pt = p, qi, pt
                        self.nkb = 4 * qi + 4
                        self.done = 0
                        self.po = ps_o.tile([17, 512], f32, tag="po", name="po")

                    def emit_upto(self, k):
                        for b in range(self.done, min(k, self.nkb)):
                            j = b - 4 * self.qi
                            qoff = 128 * j if j > 0 else 0
                            nc.tensor.matmul(
                                self.po[:, qoff:512],
                                V[self.p][:, 17 * b : 17 * b + 17],
                                self.pt[:, 512 * b + qoff : 512 * (b + 1)],
                                start=(b == 0),
                                stop=(b == self.nkb - 1),
                            )
                        self.done = max(self.done, min(k, self.nkb))

                    def finish(self):
                        self.emit_upto(self.nkb)
                        qsl = slice(512 * self.qi, 512 * (self.qi + 1))
                        ost = stg.tile([17, 512], f32, tag="ost", name="ost")
                        eng = pick(512 * ACT_NS + OVH, 512 * DVE_NS + OVHD)
                        if eng == "act":
                            nc.scalar.copy(ost[:], self.po[:])
                        else:
                            nc.vector.tensor_copy(ost[:], self.po[:])
                        nc.sync.dma_start(out=out_d[self.p][:, qsl], in_=ost[:])

                # Fine-grained software pipeline: while emitting unit i's
                # score chunks (paced by the exp engines), interleave unit
                # i-1's PV matmuls proportionally so the PE never idles.
                # big units first (pipeline ramp), tiny all-exact units
                # (qi 0/1, ACT-serial exp) tucked mid-schedule where the
                # neighbours' PV keeps the PE fed, medium unit last so the
                # drain tail is short
                units = [(p, qi) for qi in (7, 6, 5, 4, 0, 1, 3, 2) for p in range(2)]
                prev = None
                u0 = None
                for ui, (p, qi) in enumerate(units):
                    nkb = 4 * qi + 4
                    nchunks = (nkb + _CHUNK - 1) // _CHUNK
                    pt = ptp.tile([128, 512 * _NKB], bf, tag="pt", name="pt")
                    unit = BUnit(p, qi, pt)
                    if ui == 0:
                        u0 = unit
                        continue
                    if ui == 1:
                        # interleave the first two units (both qi=7, both
                        # self-paced): doubles the PE density during the
                        # pipeline ramp so the HAM clock gate stays released
                        for c in range(nchunks):
                            emit_score_chunk(u0.p, u0.qi, u0.pt, c * _CHUNK)
                            if c >= 1:
                                u0.emit_upto(c * _CHUNK)
                            emit_score_chunk(p, qi, pt, c * _CHUNK)
                            if c >= 1:
                                unit.emit_upto(c * _CHUNK)
                        u0.finish()
                        prev = unit
                        continue
                    selfpace = ui == len(units) - 1
                    for c in range(nchunks):
                        # PV batch first: gives the PE ready work while the
                        # exp of the previous chunks drains the PSUM pool
                        prev.emit_upto(((c + 1) * prev.nkb) // nchunks)
                        emit_score_chunk(p, qi, pt, c * _CHUNK)
                        if selfpace and c >= 1:
                            # last unit: self-paced PV shortens the drain
                            unit.emit_upto(c * _CHUNK)
                    prev.finish()
                    prev = unit
                prev.finish()

    nc.compile()
    return nc


def _get_nc():
    if "nc" not in _cache:
        _cache["nc"] = _build_nc()
    return _cache["nc"]


def _prepare_in_maps(x, Wq, bq, Wk, bk, Wv, bv):
    bf = ml_dtypes.bfloat16
    x = np.asarray(x, np.float32)
    Wq, bq = np.asarray(Wq, np.float32), np.asarray(bq, np.float32)
    Wk, bk = np.asarray(Wk, np.float32), np.asarray(bk, np.float32)
    Wv, bv = np.asarray(Wv, np.float32), np.asarray(bv, np.float32)
    ones = np.ones((1, _S), np.float32)

    def aug(W, b, h, scale=1.0):
        # [Dh, D+1] block for head h: weight rows plus bias column
        blk = np.concatenate(
            [W[h * _Dh : (h + 1) * _Dh, :], b[h * _Dh : (h + 1) * _Dh, None]], axis=1
        )
        return (blk * scale).T.astype(np.float32)  # [D+1, Dh]

    mask = np.where(
        np.arange(128)[:, None] > np.arange(128)[None, :], _MASK_NEG, 0.0
    ).astype(np.float32)

    in_maps = []
    for c in range(_NC):
        b_idx = c // 2
        heads = (2 * (c % 2), 2 * (c % 2) + 1)
        xT = np.concatenate([x[b_idx].T, ones], axis=0)  # [65, 4096]
        gtm = np.zeros((128, 128), np.float32)
        gbv = np.zeros((128, 1), np.float32)
        wv_cols = []
        for p, h in enumerate(heads):
            Wq_aug = aug(Wq, bq, h, _SCALE)  # [65, 16], q-scaled
            Wk_aug = aug(Wk, bk, h)  # [65, 16]
            G = Wk_aug @ Wq_aug.T  # [65, 65] = Wk_aug^T Wq_aug in row form
            gtm[64 * p : 64 * p + 64, 64 * p : 64 * p + 64] = G[0:64, 0:64].T
            gbv[64 * p : 64 * p + 64, 0] = G[0:64, 64]
            wv_cols.append(aug(Wv, bv, h))
        in_maps.append(
            {
                "xT": xT.astype(bf),
                "gt": gtm.astype(bf),
                "gb": gbv,
                "wv": np.concatenate(wv_cols, axis=1).astype(bf),
                "mask": mask,
            }
        )
    return in_maps


def _assemble(results):
    final = np.empty((_B, _S, _D), np.float32)
    for c in range(_NC):
        b_idx = c // 2
        for p in range(2):
            h = 2 * (c % 2) + p
            o = np.asarray(results[c]["out"], np.float32)  # [2, 17, S]
            final[b_idx, :, h * _Dh : (h + 1) * _Dh] = (o[p, :16] / o[p, 16:17]).T
    return final


def _run(in_maps, trace=False, trace_kwargs=None):
    from concourse.bass_utils import run_bass_kernel_spmd

    nc = _get_nc()
    return run_bass_kernel_spmd(
        nc, in_maps, list(range(_NC)), trace=trace, **(trace_kwargs or {})
    )


def kernel(x, Wq, bq, Wk, bk, Wv, bv):
    in_maps = _prepare_in_maps(x, Wq, bq, Wk, bk, Wv, bv)
    res = _run(in_maps)
    return _assemble(res.results)
